# revision 6
# baseline (speedup 1.0000x reference)
"""GNN message-passing (Convolve) kernel for Trainium2, 8 NeuronCores.

Reference computation (B=8, N=8192, C=256, H=256, O=256, K=64):
    g   = embeddings[:, neighbor_set, :]                     # [B, K, C]
    h   = leaky_relu(g @ Qw + Qb)                            # [B, K, H]
    w   = weights[neighbor_set, node_id]                     # [K]
    s   = sum_k h * w / (sum_k w + eps)                      # [B, H]
    z   = concat(embeddings[:, node_id, :], s)               # [B, C+H]
    o   = leaky_relu(z @ Ww + Wb)                            # [B, O]
    out = o / (||o||_2 + eps)                                # [B, O]

Sharding: data-parallel over the batch axis - core b handles batch b.
Each core receives an augmented bf16 table T = [embeddings[b] | weights[:,
node_id]] ([N, C+1]) so one indirect-DMA gather fetches the neighbor
embedding rows, their edge weights, AND the node's own row (node_id is
appended to the index list).  All matmul operands are bf16 (1 PE pass/row
vs fp32's 4); PSUM accumulation stays fp32.  Weights are packed host-side
into one [128, 1536] tile with 3KB-contiguous partition rows so the load
is 128 large DMA descriptors instead of 768 small ones.

Device dataflow:
    gpsimd: memset dummy idx -> 1-row dummy gather (warms the DMA library)
            -> real 65-row gather once idx lands
    sync HWDGE ring:   idx, constants [eye65 | ones]
    scalar HWDGE ring: packed weights (+ optional biases)
    PE: gT chunks via transpose; den = ones.T @ w (broadcast to 128 rows);
        xp = node.T @ Ww_top (opens PSUM group); h = gT.T @ Qw;
        s_cols = h_l.T @ w_raw; xp += (s_cols * rcp(den)).T @ Ww_bot
    ACT: h_l = Prelu(h); o = Prelu(xp); n2 = accum(Square(o)); r = Rsqrt(n2)
         (Prelu/Square/Rsqrt all live in one ACT table -> single table load,
         warmed at program start on a memset scalar)
    DVE: rcp(den + eps), z23 scale, final o * r
"""

import functools

import numpy as np
import ml_dtypes

import concourse.bacc as bacc
import concourse.bass as bass
import concourse.mybir as mybir
import concourse.tile as tile
from concourse.bass_utils import run_bass_kernel_spmd

B, N, C, H, O, K = 8, 8192, 256, 256, 256, 64
K1 = K + 1  # neighbors + the node's own row
ALPHA = 0.3
EPS = 1e-6
F32 = mybir.dt.float32
BF16 = mybir.dt.bfloat16
I32 = mybir.dt.int32
N_CORES = 8
MULT = mybir.AluOpType.mult
AF = mybir.ActivationFunctionType
BF = ml_dtypes.bfloat16


def _build_program(has_qb: bool, has_wb: bool) -> bass.Bass:
    nc = bacc.Bacc(None, target_bir_lowering=False, debug=False)

    embw = nc.dram_tensor("embw", [N, C + 1], BF16, kind="ExternalInput")
    wall = nc.dram_tensor("wall", [128, 1536], BF16, kind="ExternalInput")
    cbc = nc.dram_tensor("cbc", [K1, K1 + 128], BF16, kind="ExternalInput")
    nbr = nc.dram_tensor("nbr", [K1, 1], I32, kind="ExternalInput")
    if has_qb:
        qbd = nc.dram_tensor("qb", [1, H], BF16, kind="ExternalInput")
    if has_wb:
        wbd = nc.dram_tensor("wb", [1, O], BF16, kind="ExternalInput")
    out_d = nc.dram_tensor("out", [1, O], F32, kind="ExternalOutput")

    with tile.TileContext(nc) as tc:
        with (
            tc.tile_pool(name="sb", bufs=1) as sb,
            tc.tile_pool(name="ps", bufs=1, space="PSUM") as ps,
        ):
            # ---- gpsimd first: dummy 1-row gather starts the DMA-library
            # ifetch immediately so the real gather's descgen is warm ----
            widx = sb.tile([2, 1], I32)
            nc.gpsimd.memset(widx[:], 0)
            gw = sb.tile([2, C + 1], BF16)
            nc.gpsimd.indirect_dma_start(
                out=gw[:],
                out_offset=None,
                in_=embw[:],
                in_offset=bass.IndirectOffsetOnAxis(ap=widx[:, :1], axis=0),
            )
            # ---- sync HWDGE ring: idx first (gates gather), constants ----
            idx = sb.tile([K1, 1], I32)
            nc.sync.dma_start(out=idx[:], in_=nbr[:])
            cb = sb.tile([K1, K1 + 128], BF16)
            nc.sync.dma_start(out=cb[:], in_=cbc[:])
            # ---- scalar HWDGE ring: packed weights, optional biases ----
            wal = sb.tile([128, 1536], BF16)
            nc.scalar.dma_start(out=wal[:], in_=wall[:])
            if has_qb:
                qb_r = sb.tile([1, H], BF16)
                nc.scalar.dma_start(out=qb_r[:], in_=qbd[:])
            if has_wb:
                wb_r = sb.tile([1, O], BF16)
                nc.scalar.dma_start(out=wb_r[:], in_=wbd[:])
            # ---- warm the single ACT table (Prelu/Square/Rsqrt) on a
            # DVE-memset scalar so the table load never gates the tail ----
            warm = sb.tile([1, 1], F32)
            nc.vector.memset(warm[:], 1.0)
            w1 = sb.tile([1, 1], F32)
            nc.scalar.activation(out=w1[:], in_=warm[:], func=AF.Prelu, alpha=ALPHA)
            w2 = sb.tile([1, 1], F32)
            nc.scalar.activation(out=w2[:], in_=w1[:], func=AF.Square)
            w3 = sb.tile([1, 1], F32)
            nc.scalar.activation(out=w3[:], in_=w2[:], func=AF.Sqrt)

            # ---- the real gather: 64 neighbors + node row, [65, 257] ----
            g = sb.tile([K1, C + 1], BF16)
            nc.gpsimd.indirect_dma_start(
                out=g[:],
                out_offset=None,
                in_=embw[:],
                in_offset=bass.IndirectOffsetOnAxis(ap=idx[:, :1], axis=0),
            )

            # ---- gT chunks (embedding dims onto partitions) ----
            gt = []
            for j in range(2):
                p = ps.tile([128, K1], BF16, tag=f"t{j}")
                nc.tensor.transpose(
                    out=p[:], in_=g[:, 128 * j : 128 * (j + 1)],
                    identity=cb[:, 0:K1],
                )
                s = sb.tile([128, K1], BF16, tag=f"gts{j}")
                nc.vector.tensor_copy(out=s[:], in_=p[:])
                gt.append(s)

            # ---- den broadcast to 128 rows: ones[64,128].T @ w_col ----
            dcp = ps.tile([128, 1], F32, tag="t2")
            nc.tensor.matmul(
                out=dcp[:], lhsT=cb[0:K, K1 : K1 + 128], rhs=g[0:K, C : C + 1],
                start=True, stop=True,
            )
            dce = sb.tile([128, 1], F32)
            nc.vector.tensor_scalar_add(dce[:], dcp[:], EPS)
            rc = sb.tile([128, 1], F32)
            nc.vector.reciprocal(rc[:], dce[:])

            # ---- xp = node.T @ Ww_top (opens the PSUM group) ----
            xp = ps.tile([1, O], F32)
            nc.tensor.matmul(
                out=xp[:], lhsT=gt[0][:, K : K + 1], rhs=wal[:, 512:768],
                start=True, stop=False, skip_group_check=True,
            )
            nc.tensor.matmul(
                out=xp[:], lhsT=gt[1][:, K : K + 1], rhs=wal[:, 768:1024],
                start=False, stop=False, skip_group_check=True,
            )

            # ---- h = Prelu(gT.T @ Qw (+ Qb)) ----
            hp = ps.tile([K, H], F32)
            nc.tensor.matmul(
                out=hp[:], lhsT=gt[0][:, 0:K], rhs=wal[:, 0:256],
                start=True, stop=False,
            )
            nc.tensor.matmul(
                out=hp[:], lhsT=gt[1][:, 0:K], rhs=wal[:, 256:512],
                start=False, stop=not has_qb,
            )
            if has_qb:
                nc.tensor.matmul(
                    out=hp[:], lhsT=cb[0:1, K1 : K1 + K], rhs=qb_r[:],
                    start=False, stop=True,
                )
            h_l = sb.tile([K, H], BF16)
            nc.scalar.activation(out=h_l[:], in_=hp[:], func=AF.Prelu, alpha=ALPHA)

            # ---- s cols = h_l.T @ w_raw, scaled by rcp(den) on copy ----
            z23 = sb.tile([128, 2], BF16)
            for j in range(2):
                p = ps.tile([128, 1], F32, tag=f"t{j}")
                nc.tensor.matmul(
                    out=p[:], lhsT=h_l[:, 128 * j : 128 * (j + 1)],
                    rhs=g[0:K, C : C + 1], start=True, stop=True,
                )
                nc.vector.tensor_tensor(
                    out=z23[:, j : j + 1], in0=p[:], in1=rc[:], op=MULT
                )
            nc.tensor.matmul(
                out=xp[:], lhsT=z23[:, 0:1], rhs=wal[:, 1024:1280],
                start=False, stop=False, skip_group_check=True,
            )
            nc.tensor.matmul(
                out=xp[:], lhsT=z23[:, 1:2], rhs=wal[:, 1280:1536],
                start=False, stop=not has_wb, skip_group_check=True,
            )
            if has_wb:
                nc.tensor.matmul(
                    out=xp[:], lhsT=cb[0:1, K1 : K1 + 1], rhs=wb_r[:],
                    start=False, stop=True, skip_group_check=True,
                )

            # ---- o = Prelu(xp); out = o * Rsqrt(sum o^2) ----
            o2 = sb.tile([1, O], F32)
            nc.scalar.activation(out=o2[:], in_=xp[:], func=AF.Prelu, alpha=ALPHA)
            sq = sb.tile([1, O], F32)
            n2 = sb.tile([1, 1], F32)
            nc.scalar.activation(out=sq[:], in_=o2[:], func=AF.Square, accum_out=n2[:])
            nrm = sb.tile([1, 1], F32)
            nc.scalar.activation(out=nrm[:], in_=n2[:], func=AF.Sqrt)
            r = sb.tile([1, 1], F32)
            nc.vector.reciprocal(r[:], nrm[:])
            res = sb.tile([1, O], F32)
            nc.vector.tensor_scalar_mul(res[:], o2[:], r[:])

            nc.sync.dma_start(out=out_d[:], in_=res[:])

    nc.finalize()
    return nc


@functools.lru_cache(maxsize=4)
def _program(has_qb: bool, has_wb: bool) -> bass.Bass:
    return _build_program(has_qb, has_wb)


def _consts() -> np.ndarray:
    cb = np.zeros((K1, K1 + 128), dtype=BF)
    cb[:, 0:K1] = np.eye(K1, dtype=np.float32).astype(BF)
    cb[:, K1:] = np.float32(1.0)
    return cb


def kernel(
    embeddings: np.ndarray,
    weights: np.ndarray,
    Qw: np.ndarray,
    Qb: np.ndarray,
    Ww: np.ndarray,
    Wb: np.ndarray,
    neighbor_set: np.ndarray,
    node_id,
    _trace: bool = False,
):
    node_id = int(np.asarray(node_id))
    nbr = np.concatenate(
        [np.asarray(neighbor_set).astype(np.int32).reshape(K), [node_id]]
    ).astype(np.int32).reshape(K1, 1)
    wcol = np.asarray(weights[:, node_id], dtype=np.float32).reshape(N, 1)
    qw = np.asarray(Qw, dtype=np.float32)
    ww = np.asarray(Ww, dtype=np.float32)
    wall = np.ascontiguousarray(
        np.concatenate(
            [qw[0:128], qw[128:256], ww[0:128], ww[128:256], ww[256:384], ww[384:512]],
            axis=1,
        ).astype(BF)
    )
    qb = np.asarray(Qb, dtype=np.float32).reshape(1, H)
    wb = np.asarray(Wb, dtype=np.float32).reshape(1, O)
    has_qb = bool(np.any(qb))
    has_wb = bool(np.any(wb))
    cb = _consts()

    nc = _program(has_qb, has_wb)
    in_maps = []
    for b in range(N_CORES):
        m = {
            "embw": np.ascontiguousarray(
                np.concatenate(
                    [np.asarray(embeddings[b], dtype=np.float32), wcol], axis=1
                ).astype(BF)
            ),
            "wall": wall,
            "cbc": cb,
            "nbr": nbr,
        }
        if has_qb:
            m["qb"] = qb.astype(BF)
        if has_wb:
            m["wb"] = wb.astype(BF)
        in_maps.append(m)
    r = run_bass_kernel_spmd(nc, in_maps, list(range(N_CORES)), trace=_trace)
    out = np.stack([r.results[b]["out"][0] for b in range(N_CORES)], axis=0)
    if _trace:
        return out, r
    return out


# revision 10
# speedup vs baseline: 1.0433x; 1.0433x over previous
"""GNN message-passing (Convolve) kernel for Trainium2, 8 NeuronCores.

Reference computation (B=8, N=8192, C=256, H=256, O=256, K=64):
    g   = embeddings[:, neighbor_set, :]                     # [B, K, C]
    h   = leaky_relu(g @ Qw + Qb)                            # [B, K, H]
    w   = weights[neighbor_set, node_id]                     # [K]
    s   = sum_k h * w / (sum_k w + eps)                      # [B, H]
    z   = concat(embeddings[:, node_id, :], s)               # [B, C+H]
    o   = leaky_relu(z @ Ww + Wb)                            # [B, O]
    out = o / (||o||_2 + eps)                                # [B, O]

Sharding: data-parallel over the batch axis - core b handles batch b.
Each core receives an augmented bf16 table T = [embeddings[b] | weights[:,
node_id]] ([N, C+1]) so one indirect-DMA gather fetches the neighbor
embedding rows, their edge weights, AND the node's own row (node_id is
appended to the index list).  All matmul operands are bf16 (1 PE pass/row
vs fp32's 4); PSUM accumulation stays fp32.  Weights are packed host-side
into one [128, 1536] tile with 3KB-contiguous partition rows so the load
is 128 large DMA descriptors instead of 768 small ones.

Device dataflow:
    gpsimd: memset dummy idx -> 1-row dummy gather (warms the DMA library)
            -> real 65-row gather once idx lands
    sync HWDGE ring:   idx, constants [eye65 | ones]
    scalar HWDGE ring: packed weights (+ optional biases)
    PE: gT chunks via transpose; den = ones.T @ w (broadcast to 128 rows);
        xp = node.T @ Ww_top (opens PSUM group); h = gT.T @ Qw;
        s_cols = h_l.T @ w_raw; xp += (s_cols * rcp(den)).T @ Ww_bot
    ACT: h_l = Prelu(h); o = Prelu(xp); n2 = accum(Square(o)); r = Rsqrt(n2)
         (Prelu/Square/Rsqrt all live in one ACT table -> single table load,
         warmed at program start on a memset scalar)
    DVE: rcp(den + eps), z23 scale, final o * r
"""

import functools

import numpy as np
import ml_dtypes

import concourse.bacc as bacc
import concourse.bass as bass
import concourse.mybir as mybir
import concourse.tile as tile
from concourse.bass_utils import run_bass_kernel_spmd

B, N, C, H, O, K = 8, 8192, 256, 256, 256, 64
K1 = K + 1  # neighbors + the node's own row
ALPHA = 0.3
EPS = 1e-6
F32 = mybir.dt.float32
BF16 = mybir.dt.bfloat16
I32 = mybir.dt.int32
N_CORES = 8
MULT = mybir.AluOpType.mult
AF = mybir.ActivationFunctionType
BF = ml_dtypes.bfloat16


def _build_program(has_qb: bool, has_wb: bool) -> bass.Bass:
    nc = bacc.Bacc(None, target_bir_lowering=False, debug=False)

    embw = nc.dram_tensor("embw", [N, C + 1], BF16, kind="ExternalInput")
    wall = nc.dram_tensor("wall", [128, 1536], BF16, kind="ExternalInput")
    cbc = nc.dram_tensor("cbc", [K1, K1 + 128], BF16, kind="ExternalInput")
    nbr = nc.dram_tensor("nbr", [K1, 1], I32, kind="ExternalInput")
    if has_qb:
        qbd = nc.dram_tensor("qb", [1, H], BF16, kind="ExternalInput")
    if has_wb:
        wbd = nc.dram_tensor("wb", [1, O], BF16, kind="ExternalInput")
    out_d = nc.dram_tensor("out", [1, O], F32, kind="ExternalOutput")

    with tile.TileContext(nc) as tc:
        with (
            tc.tile_pool(name="sb", bufs=1) as sb,
            tc.tile_pool(name="ps", bufs=1, space="PSUM") as ps,
        ):
            # ---- gpsimd first: dummy 1-row gather starts the DMA-library
            # ifetch immediately so the real gather's descgen is warm ----
            widx = sb.tile([2, 1], I32)
            nc.gpsimd.memset(widx[:], 0)
            gw = sb.tile([2, C + 1], BF16)
            nc.gpsimd.indirect_dma_start(
                out=gw[:],
                out_offset=None,
                in_=embw[:],
                in_offset=bass.IndirectOffsetOnAxis(ap=widx[:, :1], axis=0),
            )
            # ---- scalar HWDGE ring: idx ONLY (first packet -> lands fast,
            # gates the gather); sync ring: constants then weights ----
            idx = sb.tile([K1, 1], I32)
            nc.scalar.dma_start(out=idx[:], in_=nbr[:])
            cb = sb.tile([K1, K1 + 128], BF16)
            nc.sync.dma_start(out=cb[:], in_=cbc[:])
            wal = sb.tile([128, 1536], BF16)
            nc.sync.dma_start(out=wal[:], in_=wall[:])
            if has_qb:
                qb_r = sb.tile([1, H], BF16)
                nc.sync.dma_start(out=qb_r[:], in_=qbd[:])
            if has_wb:
                wb_r = sb.tile([1, O], BF16)
                nc.sync.dma_start(out=wb_r[:], in_=wbd[:])
            # ---- warm the ACT table on a DVE-memset scalar; Sqrt FIRST so
            # the pass picks sqrt_and_others (covers Prelu/Square too) and
            # only one 1.3us table load is ever issued ----
            warm = sb.tile([1, 1], F32)
            nc.vector.memset(warm[:], 1.0)
            w1 = sb.tile([1, 1], F32)
            nc.scalar.activation(out=w1[:], in_=warm[:], func=AF.Sqrt)
            w2 = sb.tile([1, 1], F32)
            nc.scalar.activation(out=w2[:], in_=w1[:], func=AF.Prelu, alpha=ALPHA)
            w3 = sb.tile([1, 1], F32)
            nc.scalar.activation(out=w3[:], in_=w2[:], func=AF.Square)

            # ---- the real gather: 64 neighbors + node row, [65, 257] ----
            g = sb.tile([K1, C + 1], BF16)
            nc.gpsimd.indirect_dma_start(
                out=g[:],
                out_offset=None,
                in_=embw[:],
                in_offset=bass.IndirectOffsetOnAxis(ap=idx[:, :1], axis=0),
            )

            # ---- PE p-state warm-up: ~24 dummy matmuls on the constant
            # tile keep the PE busy while the gather is in flight, so the
            # real matmul chain runs at full clock instead of mid-pstate ----
            pw = ps.tile([K, 128], F32, tag="warm")
            for _ in range(24):
                nc.tensor.matmul(
                    out=pw[:], lhsT=cb[0:K, 0:K], rhs=cb[0:K, K1 : K1 + 128],
                    start=True, stop=True,
                )

            # ---- gT chunks (embedding dims onto partitions) ----
            gt = []
            for j in range(2):
                p = ps.tile([128, K1], BF16, tag=f"t{j}")
                nc.tensor.transpose(
                    out=p[:], in_=g[:, 128 * j : 128 * (j + 1)],
                    identity=cb[:, 0:K1],
                )
                s = sb.tile([128, K1], BF16, tag=f"gts{j}")
                nc.vector.tensor_copy(out=s[:], in_=p[:])
                gt.append(s)

            # ---- h = Prelu(gT.T @ Qw (+ Qb)) -- the critical chain, so it
            # runs first on the PE after the transposes ----
            hp = ps.tile([K, H], F32)
            nc.tensor.matmul(
                out=hp[:], lhsT=gt[0][:, 0:K], rhs=wal[:, 0:256],
                start=True, stop=False,
            )
            nc.tensor.matmul(
                out=hp[:], lhsT=gt[1][:, 0:K], rhs=wal[:, 256:512],
                start=False, stop=not has_qb,
            )
            if has_qb:
                nc.tensor.matmul(
                    out=hp[:], lhsT=cb[0:1, K1 : K1 + K], rhs=qb_r[:],
                    start=False, stop=True,
                )
            h_l = sb.tile([K, H], BF16)
            nc.scalar.activation(out=h_l[:], in_=hp[:], func=AF.Prelu, alpha=ALPHA)

            # ---- while ACT runs Prelu: den broadcast + node part of xp ----
            dcp = ps.tile([128, 1], F32, tag="t2")
            nc.tensor.matmul(
                out=dcp[:], lhsT=cb[0:K, K1 : K1 + 128], rhs=g[0:K, C : C + 1],
                start=True, stop=True,
            )
            dce = sb.tile([128, 1], F32)
            nc.vector.tensor_scalar_add(dce[:], dcp[:], EPS)
            rc = sb.tile([128, 1], F32)
            nc.vector.reciprocal(rc[:], dce[:])

            xp = ps.tile([1, O], F32)
            nc.tensor.matmul(
                out=xp[:], lhsT=gt[0][:, K : K + 1], rhs=wal[:, 512:768],
                start=True, stop=False, skip_group_check=True,
            )
            nc.tensor.matmul(
                out=xp[:], lhsT=gt[1][:, K : K + 1], rhs=wal[:, 768:1024],
                start=False, stop=False, skip_group_check=True,
            )

            # ---- s cols = h_l.T @ w_raw, scaled by rcp(den) on copy ----
            z23 = sb.tile([128, 2], BF16)
            for j in range(2):
                p = ps.tile([128, 1], F32, tag=f"t{j}")
                nc.tensor.matmul(
                    out=p[:], lhsT=h_l[:, 128 * j : 128 * (j + 1)],
                    rhs=g[0:K, C : C + 1], start=True, stop=True,
                )
                nc.vector.tensor_tensor(
                    out=z23[:, j : j + 1], in0=p[:], in1=rc[:], op=MULT
                )
            nc.tensor.matmul(
                out=xp[:], lhsT=z23[:, 0:1], rhs=wal[:, 1024:1280],
                start=False, stop=False, skip_group_check=True,
            )
            nc.tensor.matmul(
                out=xp[:], lhsT=z23[:, 1:2], rhs=wal[:, 1280:1536],
                start=False, stop=not has_wb, skip_group_check=True,
            )
            if has_wb:
                nc.tensor.matmul(
                    out=xp[:], lhsT=cb[0:1, K1 : K1 + 1], rhs=wb_r[:],
                    start=False, stop=True, skip_group_check=True,
                )

            # ---- o = Prelu(xp); out = o * Rsqrt(sum o^2) ----
            o2 = sb.tile([1, O], F32)
            nc.scalar.activation(out=o2[:], in_=xp[:], func=AF.Prelu, alpha=ALPHA)
            sq = sb.tile([1, O], F32)
            n2 = sb.tile([1, 1], F32)
            nc.scalar.activation(out=sq[:], in_=o2[:], func=AF.Square, accum_out=n2[:])
            nrm = sb.tile([1, 1], F32)
            nc.scalar.activation(out=nrm[:], in_=n2[:], func=AF.Sqrt)
            r = sb.tile([1, 1], F32)
            nc.vector.reciprocal(r[:], nrm[:])
            res = sb.tile([1, O], F32)
            nc.vector.tensor_scalar_mul(res[:], o2[:], r[:])

            nc.scalar.dma_start(out=out_d[:], in_=res[:])

    nc.finalize()
    return nc


@functools.lru_cache(maxsize=4)
def _program(has_qb: bool, has_wb: bool) -> bass.Bass:
    return _build_program(has_qb, has_wb)


def _consts() -> np.ndarray:
    cb = np.zeros((K1, K1 + 128), dtype=BF)
    cb[:, 0:K1] = np.eye(K1, dtype=np.float32).astype(BF)
    cb[:, K1:] = np.float32(1.0)
    return cb


def kernel(
    embeddings: np.ndarray,
    weights: np.ndarray,
    Qw: np.ndarray,
    Qb: np.ndarray,
    Ww: np.ndarray,
    Wb: np.ndarray,
    neighbor_set: np.ndarray,
    node_id,
    _trace: bool = False,
):
    node_id = int(np.asarray(node_id))
    nbr = np.concatenate(
        [np.asarray(neighbor_set).astype(np.int32).reshape(K), [node_id]]
    ).astype(np.int32).reshape(K1, 1)
    wcol = np.asarray(weights[:, node_id], dtype=np.float32).reshape(N, 1)
    qw = np.asarray(Qw, dtype=np.float32)
    ww = np.asarray(Ww, dtype=np.float32)
    wall = np.ascontiguousarray(
        np.concatenate(
            [qw[0:128], qw[128:256], ww[0:128], ww[128:256], ww[256:384], ww[384:512]],
            axis=1,
        ).astype(BF)
    )
    qb = np.asarray(Qb, dtype=np.float32).reshape(1, H)
    wb = np.asarray(Wb, dtype=np.float32).reshape(1, O)
    has_qb = bool(np.any(qb))
    has_wb = bool(np.any(wb))
    cb = _consts()

    nc = _program(has_qb, has_wb)
    in_maps = []
    for b in range(N_CORES):
        m = {
            "embw": np.ascontiguousarray(
                np.concatenate(
                    [np.asarray(embeddings[b], dtype=np.float32), wcol], axis=1
                ).astype(BF)
            ),
            "wall": wall,
            "cbc": cb,
            "nbr": nbr,
        }
        if has_qb:
            m["qb"] = qb.astype(BF)
        if has_wb:
            m["wb"] = wb.astype(BF)
        in_maps.append(m)
    r = run_bass_kernel_spmd(nc, in_maps, list(range(N_CORES)), trace=_trace)
    out = np.stack([r.results[b]["out"][0] for b in range(N_CORES)], axis=0)
    if _trace:
        return out, r
    return out


# revision 13
# speedup vs baseline: 1.0445x; 1.0012x over previous
"""GNN message-passing (Convolve) kernel for Trainium2, 8 NeuronCores.

Reference computation (B=8, N=8192, C=256, H=256, O=256, K=64):
    g   = embeddings[:, neighbor_set, :]                     # [B, K, C]
    h   = leaky_relu(g @ Qw + Qb)                            # [B, K, H]
    w   = weights[neighbor_set, node_id]                     # [K]
    s   = sum_k h * w / (sum_k w + eps)                      # [B, H]
    z   = concat(embeddings[:, node_id, :], s)               # [B, C+H]
    o   = leaky_relu(z @ Ww + Wb)                            # [B, O]
    out = o / (||o||_2 + eps)                                # [B, O]

Sharding: data-parallel over the batch axis - core b handles batch b.
Each core receives an augmented bf16 table T = [embeddings[b] | weights[:,
node_id]] ([N, C+1]) so one indirect-DMA gather fetches the neighbor
embedding rows, their edge weights, AND the node's own row (node_id is
appended to the index list).  All matmul operands are bf16 (1 PE pass/row
vs fp32's 4); PSUM accumulation stays fp32.  Weights are packed host-side
into one [128, 1536] tile with 3KB-contiguous partition rows so the load
is 128 large DMA descriptors instead of 768 small ones.

Device dataflow:
    gpsimd: memset dummy idx -> 1-row dummy gather (warms the DMA library)
            -> real 65-row gather once idx lands
    sync HWDGE ring:   idx, constants [eye65 | ones]
    scalar HWDGE ring: packed weights (+ optional biases)
    PE: gT chunks via transpose; den = ones.T @ w (broadcast to 128 rows);
        xp = node.T @ Ww_top (opens PSUM group); h = gT.T @ Qw;
        s_cols = h_l.T @ w_raw; xp += (s_cols * rcp(den)).T @ Ww_bot
    ACT: h_l = Prelu(h); o = Prelu(xp); n2 = accum(Square(o)); r = Rsqrt(n2)
         (Prelu/Square/Rsqrt all live in one ACT table -> single table load,
         warmed at program start on a memset scalar)
    DVE: rcp(den + eps), z23 scale, final o * r
"""

import functools

import numpy as np
import ml_dtypes

import concourse.bacc as bacc
import concourse.bass as bass
import concourse.mybir as mybir
import concourse.tile as tile
from concourse.bass_utils import run_bass_kernel_spmd

B, N, C, H, O, K = 8, 8192, 256, 256, 256, 64
K1 = K + 1  # neighbors + the node's own row
ALPHA = 0.3
EPS = 1e-6
F32 = mybir.dt.float32
BF16 = mybir.dt.bfloat16
I32 = mybir.dt.int32
N_CORES = 8
MULT = mybir.AluOpType.mult
AF = mybir.ActivationFunctionType
BF = ml_dtypes.bfloat16


def _build_program(has_qb: bool, has_wb: bool) -> bass.Bass:
    nc = bacc.Bacc(None, target_bir_lowering=False, debug=False)

    embw = nc.dram_tensor("embw", [N, C + 1], BF16, kind="ExternalInput")
    wall = nc.dram_tensor("wall", [128, 1536], BF16, kind="ExternalInput")
    cbc = nc.dram_tensor("cbc", [K1, K1 + 128], BF16, kind="ExternalInput")
    nbr = nc.dram_tensor("nbr", [K1, 1], I32, kind="ExternalInput")
    if has_qb:
        qbd = nc.dram_tensor("qb", [1, H], BF16, kind="ExternalInput")
    if has_wb:
        wbd = nc.dram_tensor("wb", [1, O], BF16, kind="ExternalInput")
    out_d = nc.dram_tensor("out", [1, O], F32, kind="ExternalOutput")

    with tile.TileContext(nc) as tc:
        with (
            tc.tile_pool(name="sb", bufs=1) as sb,
            tc.tile_pool(name="ps", bufs=1, space="PSUM") as ps,
        ):
            # ---- gpsimd first: dummy 1-row gather starts the DMA-library
            # ifetch immediately so the real gather's descgen is warm ----
            widx = sb.tile([2, 1], I32)
            nc.gpsimd.memset(widx[:], 0)
            gw = sb.tile([2, C + 1], BF16)
            nc.gpsimd.indirect_dma_start(
                out=gw[:],
                out_offset=None,
                in_=embw[:],
                in_offset=bass.IndirectOffsetOnAxis(ap=widx[:, :1], axis=0),
            )
            # ---- sync HWDGE ring carries ALL loads in priority order: idx
            # (gates the gather), constants, Qw, then Ww.  The scalar ring
            # (q10) carries only the auto-inserted ACT table loads, so they
            # can't delay idx, and q10's SDMA priority can't starve q1 ----
            idx = sb.tile([K1, 1], I32)
            nc.sync.dma_start(out=idx[:], in_=nbr[:])
            cb = sb.tile([K1, K1 + 128], BF16)
            nc.sync.dma_start(out=cb[:], in_=cbc[:])
            wal = sb.tile([128, 1536], BF16)
            nc.sync.dma_start(out=wal[:, 0:512], in_=wall[:, 0:512])
            nc.sync.dma_start(out=wal[:, 512:1536], in_=wall[:, 512:1536])
            if has_qb:
                qb_r = sb.tile([1, H], BF16)
                nc.sync.dma_start(out=qb_r[:], in_=qbd[:])
            if has_wb:
                wb_r = sb.tile([1, O], BF16)
                nc.sync.dma_start(out=wb_r[:], in_=wbd[:])
            # ---- warm the ACT tables (Sqrt + Prelu/Square) on a DVE-memset
            # scalar so both table loads happen at program start ----
            warm = sb.tile([1, 1], F32)
            nc.vector.memset(warm[:], 1.0)
            w1 = sb.tile([1, 1], F32)
            nc.scalar.activation(out=w1[:], in_=warm[:], func=AF.Sqrt)
            w2 = sb.tile([1, 1], F32)
            nc.scalar.activation(out=w2[:], in_=w1[:], func=AF.Prelu, alpha=ALPHA)
            w3 = sb.tile([1, 1], F32)
            nc.scalar.activation(out=w3[:], in_=w2[:], func=AF.Square)

            # ---- the real gather: 64 neighbors + node row, [65, 257] ----
            g = sb.tile([K1, C + 1], BF16)
            nc.gpsimd.indirect_dma_start(
                out=g[:],
                out_offset=None,
                in_=embw[:],
                in_offset=bass.IndirectOffsetOnAxis(ap=idx[:, :1], axis=0),
            )

            # ---- gT chunks (embedding dims onto partitions) ----
            gt = []
            for j in range(2):
                p = ps.tile([128, K1], BF16, tag=f"t{j}")
                nc.tensor.transpose(
                    out=p[:], in_=g[:, 128 * j : 128 * (j + 1)],
                    identity=cb[:, 0:K1],
                )
                s = sb.tile([128, K1], BF16, tag=f"gts{j}")
                nc.vector.tensor_copy(out=s[:], in_=p[:])
                gt.append(s)

            # ---- h = Prelu(gT.T @ Qw (+ Qb)), split into column halves so
            # Prelu(half A) runs on ACT while the PE computes half B; each
            # s-col j only needs its own half ----
            hps, hls = [], []
            for j in range(2):
                hp = ps.tile([K, 128], F32, tag=f"h{j}")
                nc.tensor.matmul(
                    out=hp[:], lhsT=gt[0][:, 0:K],
                    rhs=wal[:, 128 * j : 128 * (j + 1)],
                    start=True, stop=False,
                )
                nc.tensor.matmul(
                    out=hp[:], lhsT=gt[1][:, 0:K],
                    rhs=wal[:, 256 + 128 * j : 256 + 128 * (j + 1)],
                    start=False, stop=not has_qb,
                )
                if has_qb:
                    nc.tensor.matmul(
                        out=hp[:], lhsT=cb[0:1, K1 : K1 + K],
                        rhs=qb_r[:, 128 * j : 128 * (j + 1)],
                        start=False, stop=True,
                    )
                hl = sb.tile([K, 128], BF16, tag=f"hl{j}")
                nc.scalar.activation(out=hl[:], in_=hp[:], func=AF.Prelu, alpha=ALPHA)
                hps.append(hp)
                hls.append(hl)

            # ---- while ACT runs Prelu: den broadcast + node part of xp ----
            dcp = ps.tile([128, 1], F32, tag="t2")
            nc.tensor.matmul(
                out=dcp[:], lhsT=cb[0:K, K1 : K1 + 128], rhs=g[0:K, C : C + 1],
                start=True, stop=True,
            )
            dce = sb.tile([128, 1], F32)
            nc.vector.tensor_scalar_add(dce[:], dcp[:], EPS)
            rc = sb.tile([128, 1], F32)
            nc.vector.reciprocal(rc[:], dce[:])

            xp = ps.tile([1, O], F32)
            nc.tensor.matmul(
                out=xp[:], lhsT=gt[0][:, K : K + 1], rhs=wal[:, 512:768],
                start=True, stop=False, skip_group_check=True,
            )
            nc.tensor.matmul(
                out=xp[:], lhsT=gt[1][:, K : K + 1], rhs=wal[:, 768:1024],
                start=False, stop=False, skip_group_check=True,
            )

            # ---- s cols = h_l.T @ w_raw, scaled by rcp(den) on copy ----
            z23 = sb.tile([128, 2], BF16)
            for j in range(2):
                p = ps.tile([128, 1], F32, tag=f"t{j}")
                nc.tensor.matmul(
                    out=p[:], lhsT=hls[j][:], rhs=g[0:K, C : C + 1],
                    start=True, stop=True,
                )
                nc.vector.tensor_tensor(
                    out=z23[:, j : j + 1], in0=p[:], in1=rc[:], op=MULT
                )
            nc.tensor.matmul(
                out=xp[:], lhsT=z23[:, 0:1], rhs=wal[:, 1024:1280],
                start=False, stop=False, skip_group_check=True,
            )
            nc.tensor.matmul(
                out=xp[:], lhsT=z23[:, 1:2], rhs=wal[:, 1280:1536],
                start=False, stop=not has_wb, skip_group_check=True,
            )
            if has_wb:
                nc.tensor.matmul(
                    out=xp[:], lhsT=cb[0:1, K1 : K1 + 1], rhs=wb_r[:],
                    start=False, stop=True, skip_group_check=True,
                )

            # ---- o = Prelu(xp); out = o * Rsqrt(sum o^2) ----
            o2 = sb.tile([1, O], F32)
            nc.scalar.activation(out=o2[:], in_=xp[:], func=AF.Prelu, alpha=ALPHA)
            sq = sb.tile([1, O], F32)
            n2 = sb.tile([1, 1], F32)
            nc.scalar.activation(out=sq[:], in_=o2[:], func=AF.Square, accum_out=n2[:])
            nrm = sb.tile([1, 1], F32)
            nc.scalar.activation(out=nrm[:], in_=n2[:], func=AF.Sqrt)
            r = sb.tile([1, 1], F32)
            nc.vector.reciprocal(r[:], nrm[:])
            res = sb.tile([1, O], F32)
            nc.vector.tensor_scalar_mul(res[:], o2[:], r[:])

            nc.scalar.dma_start(out=out_d[:], in_=res[:])

    nc.finalize()
    return nc


@functools.lru_cache(maxsize=4)
def _program(has_qb: bool, has_wb: bool) -> bass.Bass:
    return _build_program(has_qb, has_wb)


def _consts() -> np.ndarray:
    cb = np.zeros((K1, K1 + 128), dtype=BF)
    cb[:, 0:K1] = np.eye(K1, dtype=np.float32).astype(BF)
    cb[:, K1:] = np.float32(1.0)
    return cb


def kernel(
    embeddings: np.ndarray,
    weights: np.ndarray,
    Qw: np.ndarray,
    Qb: np.ndarray,
    Ww: np.ndarray,
    Wb: np.ndarray,
    neighbor_set: np.ndarray,
    node_id,
    _trace: bool = False,
):
    node_id = int(np.asarray(node_id))
    nbr = np.concatenate(
        [np.asarray(neighbor_set).astype(np.int32).reshape(K), [node_id]]
    ).astype(np.int32).reshape(K1, 1)
    wcol = np.asarray(weights[:, node_id], dtype=np.float32).reshape(N, 1)
    qw = np.asarray(Qw, dtype=np.float32)
    ww = np.asarray(Ww, dtype=np.float32)
    wall = np.ascontiguousarray(
        np.concatenate(
            [qw[0:128], qw[128:256], ww[0:128], ww[128:256], ww[256:384], ww[384:512]],
            axis=1,
        ).astype(BF)
    )
    qb = np.asarray(Qb, dtype=np.float32).reshape(1, H)
    wb = np.asarray(Wb, dtype=np.float32).reshape(1, O)
    has_qb = bool(np.any(qb))
    has_wb = bool(np.any(wb))
    cb = _consts()

    nc = _program(has_qb, has_wb)
    in_maps = []
    for b in range(N_CORES):
        m = {
            "embw": np.ascontiguousarray(
                np.concatenate(
                    [np.asarray(embeddings[b], dtype=np.float32), wcol], axis=1
                ).astype(BF)
            ),
            "wall": wall,
            "cbc": cb,
            "nbr": nbr,
        }
        if has_qb:
            m["qb"] = qb.astype(BF)
        if has_wb:
            m["wb"] = wb.astype(BF)
        in_maps.append(m)
    r = run_bass_kernel_spmd(nc, in_maps, list(range(N_CORES)), trace=_trace)
    out = np.stack([r.results[b]["out"][0] for b in range(N_CORES)], axis=0)
    if _trace:
        return out, r
    return out
